# revision 1
# baseline (speedup 1.0000x reference)
"""Trainium2 Bass kernel for nn_Attn_69801808495303.

Computes, for encoder_outputs [L, B, 2H], W [H, 2H], b [H], v [H, 1]:
    energy = tanh(enc @ W.T + b)          # [L, B, H]
    scores = energy @ v                   # [L, B]
    attn   = softmax over B (per (L, f))  # broadcast over num_features
    out    = attn as [B, num_features, L]

Strategy: shard over L across 8 NeuronCores (embarrassingly parallel,
softmax-over-batch is local to every L row). Host pre-transposes the
encoder shard to [2H, L_loc*B] bf16 so the contraction dim lands on SBUF
partitions; W/b/v are replicated. Device does the GEMM in bf16 on the
TensorEngine (W stationary), fused tanh+bias on ScalarE, the v-matvec as
M=1 matmuls, and the 64-wide batch softmax on ScalarE/VectorE. Each core
returns its [L_loc, B] probability block; the host concatenates and
broadcasts over num_features.
"""

import sys

for _p in ("/opt/trn_rl_repo", "/opt/pypackages"):
    if _p not in sys.path:
        sys.path.append(_p)

import numpy as np
import ml_dtypes

L, B, H, D = 2048, 64, 1024, 2048  # D = 2H
N_CORES = 8
L_LOC = L // N_CORES        # 256 rows of L per core
M = L_LOC * B               # 16384 tokens per core
M_BLK = 512
N_BLKS = M // M_BLK         # 32
D_TILES = D // 128          # 16
H_TILES = H // 128          # 8
GROUPS = L_LOC // 128       # 2 softmax row-groups per partition pass

BF16 = ml_dtypes.bfloat16

_compiled = {}
LAST_RESULTS = None


def _build():
    import concourse.mybir as mybir
    import concourse.tile as tile
    from concourse import bacc

    fp32, bf16 = mybir.dt.float32, mybir.dt.bfloat16
    AF = mybir.ActivationFunctionType

    nc = bacc.Bacc("TRN2", target_bir_lowering=False, debug=False,
                   num_devices=N_CORES)

    encT = nc.dram_tensor("encT", [D, M], bf16, kind="ExternalInput").ap()
    wt = nc.dram_tensor("wt", [D, H], bf16, kind="ExternalInput").ap()
    bT = nc.dram_tensor("bT", [128, H_TILES], fp32, kind="ExternalInput").ap()
    vT = nc.dram_tensor("vT", [128, H_TILES], bf16, kind="ExternalInput").ap()
    out = nc.dram_tensor("out", [L_LOC, B], fp32, kind="ExternalOutput").ap()

    encT_t = encT.rearrange("(dt p) m -> p dt m", p=128)  # [128, D_TILES, M]
    wt_t = wt.rearrange("(dt p) h -> p dt h", p=128)      # [128, D_TILES, H]

    with tile.TileContext(nc) as tc:
        with (
            tc.tile_pool(name="const", bufs=1) as cpool,
            tc.tile_pool(name="enc", bufs=3) as epool,
            tc.tile_pool(name="energy", bufs=4) as gpool,
            tc.tile_pool(name="misc", bufs=1) as mpool,
            tc.tile_pool(name="psum_e", bufs=4, space="PSUM") as pe_pool,
            tc.tile_pool(name="psum_s", bufs=2, space="PSUM") as ps_pool,
            tc.tile_pool(name="dram", bufs=1, space="DRAM") as dpool,
        ):
            wt_sb = cpool.tile([128, D_TILES, H], bf16)
            for i in range(D_TILES):
                nc.sync.dma_start(wt_sb[:, i:i + 1, :], wt_t[:, i:i + 1, :])
            b_sb = cpool.tile([128, H_TILES], fp32)
            nc.sync.dma_start(b_sb[:], bT[:])
            v_sb = cpool.tile([128, H_TILES], bf16)
            nc.sync.dma_start(v_sb[:], vT[:])

            scores_sb = mpool.tile([1, M], fp32)

            for mb in range(N_BLKS):
                msl = slice(mb * M_BLK, (mb + 1) * M_BLK)
                et = epool.tile([128, D_TILES, M_BLK], bf16, tag="enc")
                for i in range(8):
                    nc.sync.dma_start(
                        et[:, 2 * i:2 * i + 2, :],
                        encT_t[:, 2 * i:2 * i + 2, msl])
                score_ps = ps_pool.tile([1, M_BLK], fp32, tag="score")
                for ht in range(H_TILES):
                    pe = pe_pool.tile([128, M_BLK], fp32, tag="epsum")
                    for dt in range(D_TILES):
                        nc.tensor.matmul(
                            pe[:],
                            wt_sb[:, dt, ht * 128:(ht + 1) * 128],
                            et[:, dt, :],
                            start=(dt == 0), stop=(dt == D_TILES - 1))
                    eng = gpool.tile([128, M_BLK], bf16, tag="energy")
                    nc.scalar.activation(eng[:], pe[:], AF.Tanh,
                                         bias=b_sb[:, ht:ht + 1])
                    nc.tensor.matmul(score_ps[:], v_sb[:, ht:ht + 1], eng[:],
                                     start=(ht == 0), stop=(ht == H_TILES - 1))
                nc.vector.tensor_copy(scores_sb[:, msl], score_ps[:])

            # Batch softmax: m = l_loc*64 + b, so every consecutive run of 64
            # scores is one softmax group. Bounce through DRAM to regroup as
            # [128 partitions, GROUPS, B] with l = GROUPS*p + g.
            sc_dram = dpool.tile([1, M], fp32)
            nc.sync.dma_start(sc_dram[:], scores_sb[:])
            sc2 = mpool.tile([128, GROUPS, B], fp32)
            nc.sync.dma_start(
                sc2[:], sc_dram.rearrange("o (p g c) -> (o p) g c", p=128,
                                          g=GROUPS))
            negmax = mpool.tile([128, GROUPS], fp32)
            nc.vector.reduce_max(negmax[:], sc2[:], axis=mybir.AxisListType.X)
            nc.vector.tensor_scalar_mul(negmax[:], negmax[:], -1.0)
            probs = mpool.tile([128, GROUPS, B], fp32)
            sums = mpool.tile([128, GROUPS], fp32)
            for g in range(GROUPS):
                nc.scalar.activation(probs[:, g, :], sc2[:, g, :], AF.Exp,
                                     bias=negmax[:, g:g + 1],
                                     accum_out=sums[:, g:g + 1])
            rsum = mpool.tile([128, GROUPS], fp32)
            nc.vector.reciprocal(rsum[:], sums[:])
            for g in range(GROUPS):
                nc.vector.tensor_scalar_mul(probs[:, g, :], probs[:, g, :],
                                            rsum[:, g:g + 1])
            nc.sync.dma_start(out.rearrange("(p g) c -> p g c", g=GROUPS),
                              probs[:])

    nc.compile()
    return nc


def kernel(num_features, encoder_outputs, W, b, v):
    global LAST_RESULTS
    from concourse.bass_utils import run_bass_kernel_spmd

    enc = np.asarray(encoder_outputs, dtype=np.float32)
    W_np = np.asarray(W, dtype=np.float32)
    b_np = np.asarray(b, dtype=np.float32)
    v_np = np.asarray(v, dtype=np.float32)
    F = int(np.asarray(num_features))
    assert enc.shape == (L, B, D) and W_np.shape == (H, D)

    wt_np = np.ascontiguousarray(W_np.T).astype(BF16)              # [D, H]
    bT_np = np.ascontiguousarray(b_np.reshape(H_TILES, 128).T)     # [128, 8]
    vT_np = np.ascontiguousarray(
        v_np.ravel().reshape(H_TILES, 128).T).astype(BF16)         # [128, 8]

    in_maps = []
    for c in range(N_CORES):
        shard = enc[c * L_LOC:(c + 1) * L_LOC].reshape(M, D).astype(BF16)
        encT_np = np.ascontiguousarray(shard.T)                    # [D, M]
        in_maps.append({"encT": encT_np, "wt": wt_np, "bT": bT_np,
                        "vT": vT_np})

    if "nc" not in _compiled:
        _compiled["nc"] = _build()
    nc = _compiled["nc"]

    res = run_bass_kernel_spmd(nc, in_maps, core_ids=list(range(N_CORES)))
    LAST_RESULTS = res

    probs = np.concatenate([res.results[c]["out"] for c in range(N_CORES)],
                           axis=0)                                 # [L, B]
    out = np.broadcast_to(probs.T[:, None, :], (B, F, L))
    return np.ascontiguousarray(out)


# revision 2
# speedup vs baseline: 1.1199x; 1.1199x over previous
"""Trainium2 Bass kernel for nn_Attn_69801808495303.

Computes, for encoder_outputs [L, B, 2H], W [H, 2H], b [H], v [H, 1]:
    energy = tanh(enc @ W.T + b)          # [L, B, H]
    scores = energy @ v                   # [L, B]
    attn   = softmax over B (per (L, f))  # broadcast over num_features
    out    = attn as [B, num_features, L]

Strategy: shard over L across 8 NeuronCores (embarrassingly parallel —
the softmax over batch is local to every L row). Host pre-transposes the
encoder shard to [2H, L_loc*B] bf16 so the contraction dim lands on SBUF
partitions; W/b/v are replicated. On device the TensorEngine runs only
the bf16 GEMM (W stationary, energy.T [h, m] tiles in PSUM); ScalarE
applies tanh+bias and the per-partition *v scale; VectorE sums the 8
h-tiles; GpSimd reduces over partitions to finish scores = v.tanh(...);
the 64-wide batch softmax runs in two halves so most of it hides under
the GEMM. Each core returns its [L_loc, B] probability block; the host
concatenates and broadcasts over num_features.
"""

import sys

for _p in ("/opt/trn_rl_repo", "/opt/pypackages"):
    if _p not in sys.path:
        sys.path.append(_p)

import numpy as np
import ml_dtypes

L, B, H, D = 2048, 64, 1024, 2048  # D = 2H
N_CORES = 8
L_LOC = L // N_CORES        # 256 rows of L per core
M = L_LOC * B               # 16384 tokens per core
M_BLK = 512
N_BLKS = M // M_BLK         # 32
D_TILES = D // 128          # 16
H_TILES = H // 128          # 8

BF16 = ml_dtypes.bfloat16

_compiled = {}
LAST_RESULTS = None


def _build():
    import concourse.mybir as mybir
    import concourse.tile as tile
    from concourse import bacc, bass_isa

    fp32, bf16 = mybir.dt.float32, mybir.dt.bfloat16
    AF = mybir.ActivationFunctionType

    nc = bacc.Bacc("TRN2", target_bir_lowering=False, debug=False,
                   num_devices=N_CORES)

    encT = nc.dram_tensor("encT", [D, M], bf16, kind="ExternalInput").ap()
    wt = nc.dram_tensor("wt", [D, H], bf16, kind="ExternalInput").ap()
    bT = nc.dram_tensor("bT", [128, H_TILES], fp32, kind="ExternalInput").ap()
    vT = nc.dram_tensor("vT", [128, H_TILES], fp32, kind="ExternalInput").ap()
    out = nc.dram_tensor("out", [L_LOC, B], fp32, kind="ExternalOutput").ap()

    encT_t = encT.rearrange("(dt p) m -> p dt m", p=128)  # [128, D_TILES, M]
    wt_t = wt.rearrange("(dt p) h -> p dt h", p=128)      # [128, D_TILES, H]

    with tile.TileContext(nc) as tc:
        with (
            tc.tile_pool(name="const", bufs=1) as cpool,
            tc.tile_pool(name="enc", bufs=3) as epool,
            tc.tile_pool(name="eng", bufs=4) as gpool,
            tc.tile_pool(name="veng", bufs=2) as vpool,
            tc.tile_pool(name="accp", bufs=3) as apool,
            tc.tile_pool(name="misc", bufs=1) as mpool,
            tc.tile_pool(name="psum_e", bufs=6, space="PSUM") as pe_pool,
            tc.tile_pool(name="dram", bufs=1, space="DRAM") as dpool,
        ):
            wt_sb = cpool.tile([128, D_TILES, H], bf16)
            for i in range(D_TILES):
                nc.sync.dma_start(wt_sb[:, i:i + 1, :], wt_t[:, i:i + 1, :])
            b_sb = cpool.tile([128, H_TILES], fp32)
            nc.sync.dma_start(b_sb[:], bT[:])
            v_sb = cpool.tile([128, H_TILES], fp32)
            nc.sync.dma_start(v_sb[:], vT[:])

            sc_dram = dpool.tile([1, M], fp32)

            def softmax_half(half):
                """Softmax over 64-wide batch groups for one half of the
                scores (m in [half*M/2, (half+1)*M/2) => partitions
                [half*64, half*64+64) of the regrouped view)."""
                P2 = 64  # partitions per half
                sc2 = mpool.tile([P2, 2, B], fp32, tag="sc2",
                                 name=f"sc2_{half}")
                src = sc_dram.rearrange("o (p g c) -> (o p) g c", p=128, g=2)
                nc.sync.dma_start(sc2[:], src[half * P2:(half + 1) * P2])
                probs = mpool.tile([P2, 2, B], fp32, tag="probs",
                                   name=f"probs_{half}")
                sums = mpool.tile([P2, 2], fp32, tag="sums",
                                  name=f"sums_{half}")
                for g in range(2):
                    nc.scalar.activation(probs[:, g, :], sc2[:, g, :], AF.Exp,
                                         accum_out=sums[:, g:g + 1])
                rsum = mpool.tile([P2, 2], fp32, tag="rsum",
                                  name=f"rsum_{half}")
                nc.vector.reciprocal(rsum[:], sums[:])
                for g in range(2):
                    nc.vector.tensor_scalar_mul(probs[:, g, :], probs[:, g, :],
                                                rsum[:, g:g + 1])
                dst = out.rearrange("(p g) c -> p g c", g=2)
                nc.sync.dma_start(dst[half * P2:(half + 1) * P2], probs[:])

            for mb in range(N_BLKS):
                msl = slice(mb * M_BLK, (mb + 1) * M_BLK)
                et = epool.tile([128, D_TILES, M_BLK], bf16, tag="enc")
                for i in range(8):
                    nc.sync.dma_start(
                        et[:, 2 * i:2 * i + 2, :],
                        encT_t[:, 2 * i:2 * i + 2, msl])
                veng = vpool.tile([128, H_TILES, M_BLK], fp32, tag="veng")
                for ht in range(H_TILES):
                    pe = pe_pool.tile([128, M_BLK], fp32, tag="epsum")
                    for dt in range(D_TILES):
                        nc.tensor.matmul(
                            pe[:],
                            wt_sb[:, dt, ht * 128:(ht + 1) * 128],
                            et[:, dt, :],
                            start=(dt == 0), stop=(dt == D_TILES - 1))
                    eng = gpool.tile([128, M_BLK], fp32, tag="eng")
                    nc.scalar.activation(eng[:], pe[:], AF.Tanh,
                                         bias=b_sb[:, ht:ht + 1])
                    nc.scalar.mul(veng[:, ht, :], eng[:], v_sb[:, ht:ht + 1])
                # Sum the 8 h-tiles with an add tree on VectorE.
                s4 = apool.tile([128, 4, M_BLK], fp32, tag="s4")
                for i in range(4):
                    nc.vector.tensor_add(s4[:, i, :], veng[:, 2 * i, :],
                                         veng[:, 2 * i + 1, :])
                s2 = apool.tile([128, 2, M_BLK], fp32, tag="s2")
                for i in range(2):
                    nc.vector.tensor_add(s2[:, i, :], s4[:, 2 * i, :],
                                         s4[:, 2 * i + 1, :])
                acc = apool.tile([128, M_BLK], fp32, tag="acc")
                nc.vector.tensor_add(acc[:], s2[:, 0, :], s2[:, 1, :])
                # Reduce over partitions: scores[m] = sum_h v.h tanh(.)
                red = apool.tile([128, M_BLK], fp32, tag="red")
                nc.gpsimd.partition_all_reduce(red[:], acc[:], 128,
                                               bass_isa.ReduceOp.add)
                nc.sync.dma_start(sc_dram[:, msl], red[0:1, :])
                if mb == N_BLKS // 2 - 1:
                    softmax_half(0)
            softmax_half(1)

    nc.compile()
    return nc


def kernel(num_features, encoder_outputs, W, b, v):
    global LAST_RESULTS
    from concourse.bass_utils import run_bass_kernel_spmd

    enc = np.asarray(encoder_outputs, dtype=np.float32)
    W_np = np.asarray(W, dtype=np.float32)
    b_np = np.asarray(b, dtype=np.float32)
    v_np = np.asarray(v, dtype=np.float32)
    F = int(np.asarray(num_features))
    assert enc.shape == (L, B, D) and W_np.shape == (H, D)

    wt_np = np.ascontiguousarray(W_np.T).astype(BF16)              # [D, H]
    bT_np = np.ascontiguousarray(b_np.reshape(H_TILES, 128).T)     # [128, 8]
    vT_np = np.ascontiguousarray(v_np.ravel().reshape(H_TILES, 128).T)

    in_maps = []
    for c in range(N_CORES):
        shard = enc[c * L_LOC:(c + 1) * L_LOC].reshape(M, D).astype(BF16)
        encT_np = np.ascontiguousarray(shard.T)                    # [D, M]
        in_maps.append({"encT": encT_np, "wt": wt_np, "bT": bT_np,
                        "vT": vT_np})

    if "nc" not in _compiled:
        _compiled["nc"] = _build()
    nc = _compiled["nc"]

    res = run_bass_kernel_spmd(nc, in_maps, core_ids=list(range(N_CORES)))
    LAST_RESULTS = res

    probs = np.concatenate([res.results[c]["out"] for c in range(N_CORES)],
                           axis=0)                                 # [L, B]
    out = np.broadcast_to(probs.T[:, None, :], (B, F, L))
    return np.ascontiguousarray(out)


# revision 3
# speedup vs baseline: 1.3506x; 1.2060x over previous
"""Trainium2 Bass kernel for nn_Attn_69801808495303.

Computes, for encoder_outputs [L, B, 2H], W [H, 2H], b [H], v [H, 1]:
    energy = tanh(enc @ W.T + b)          # [L, B, H]
    scores = energy @ v                   # [L, B]
    attn   = softmax over B (per (L, f))  # broadcast over num_features
    out    = attn as [B, num_features, L]

Strategy: shard over L across 8 NeuronCores (embarrassingly parallel —
the softmax over batch is local to every L row). Host pre-transposes the
encoder shard to [2H, L_loc*B] bf16 so the contraction dim lands on SBUF
partitions; W/b/v are replicated. On device the TensorEngine runs only
the bf16 GEMM (W stationary, energy.T [h, m] tiles in PSUM); ScalarE
applies tanh+bias and the per-partition *v scale; VectorE accumulates the
8 h-tiles; GpSimd reduces over partitions to finish scores = v.tanh(...);
the 64-wide batch softmax runs in quarters so it hides under the GEMM.
Each core returns its [L_loc, B] probability block; the host concatenates
and broadcasts over num_features.
"""

import sys

for _p in ("/opt/trn_rl_repo", "/opt/pypackages"):
    if _p not in sys.path:
        sys.path.append(_p)

import numpy as np
import ml_dtypes

L, B, H, D = 2048, 64, 1024, 2048  # D = 2H
N_CORES = 8
L_LOC = L // N_CORES        # 256 rows of L per core
M = L_LOC * B               # 16384 tokens per core
M_BLK = 512
N_BLKS = M // M_BLK         # 32
D_TILES = D // 128          # 16
H_TILES = H // 128          # 8

BF16 = ml_dtypes.bfloat16

_compiled = {}
LAST_RESULTS = None


def _build():
    import concourse.mybir as mybir
    import concourse.tile as tile
    from concourse import bacc, bass_isa

    fp32, bf16 = mybir.dt.float32, mybir.dt.bfloat16
    AF = mybir.ActivationFunctionType

    nc = bacc.Bacc("TRN2", target_bir_lowering=False, debug=False,
                   num_devices=N_CORES)

    encT = nc.dram_tensor("encT", [D, M], bf16, kind="ExternalInput").ap()
    # weights pre-grouped by h-tile: wr[ht, d, j] = W[ht*128+j, d]
    wr = nc.dram_tensor("wr", [H_TILES, D, 128], bf16,
                        kind="ExternalInput").ap()
    bT = nc.dram_tensor("bT", [128, H_TILES], fp32, kind="ExternalInput").ap()
    vT = nc.dram_tensor("vT", [128, H_TILES], fp32, kind="ExternalInput").ap()
    out = nc.dram_tensor("out", [L_LOC, B], fp32, kind="ExternalOutput").ap()

    encT_t = encT.rearrange("(dt p) m -> p dt m", p=128)  # [128, D_TILES, M]
    wr_t = wr.rearrange("ht (dt p) j -> ht p dt j", p=128)

    with tile.TileContext(nc) as tc:
        with (
            tc.tile_pool(name="const", bufs=1) as cpool,
            tc.tile_pool(name="enc", bufs=4) as epool,
            tc.tile_pool(name="eng", bufs=4) as gpool,
            tc.tile_pool(name="veng", bufs=16) as vpool,
            tc.tile_pool(name="accp", bufs=3) as apool,
            tc.tile_pool(name="misc", bufs=2) as mpool,
            tc.tile_pool(name="psum_e", bufs=6, space="PSUM") as pe_pool,
            tc.tile_pool(name="dram", bufs=1, space="DRAM") as dpool,
        ):
            # Interleave the first et block's chunks with the weight DMAs so
            # the first matmuls start as soon as possible.
            wt_sb = [cpool.tile([128, D_TILES, 128], bf16, name=f"wt{ht}")
                     for ht in range(H_TILES)]
            et0 = epool.tile([128, D_TILES, M_BLK], bf16, tag="enc",
                             name="et0")
            nc.sync.dma_start(wt_sb[0][:, 0:8, :], wr_t[0, :, 0:8, :])
            nc.sync.dma_start(wt_sb[0][:, 8:16, :], wr_t[0, :, 8:16, :])
            for i in range(8):
                nc.sync.dma_start(et0[:, 2 * i:2 * i + 2, :],
                                  encT_t[:, 2 * i:2 * i + 2, 0:M_BLK])
            for ht in range(1, H_TILES):
                nc.sync.dma_start(wt_sb[ht][:, 0:8, :], wr_t[ht, :, 0:8, :])
                nc.sync.dma_start(wt_sb[ht][:, 8:16, :], wr_t[ht, :, 8:16, :])
            b_sb = cpool.tile([128, H_TILES], fp32)
            nc.sync.dma_start(b_sb[:], bT[:])
            v_sb = cpool.tile([128, H_TILES], fp32)
            nc.sync.dma_start(v_sb[:], vT[:])

            sc_dram = dpool.tile([1, M], fp32)

            def softmax_part(part, nparts):
                """Softmax over 64-wide batch groups for one 1/nparts slice
                of the scores (partitions [part*PP, (part+1)*PP) of the
                [128, 2, B] regrouped view)."""
                PP = 128 // nparts
                sc2 = mpool.tile([PP, 2, B], fp32, tag="sc2",
                                 name=f"sc2_{part}")
                src = sc_dram.rearrange("o (p g c) -> (o p) g c", p=128, g=2)
                nc.sync.dma_start(sc2[:], src[part * PP:(part + 1) * PP])
                probs = mpool.tile([PP, 2, B], fp32, tag="probs",
                                   name=f"probs_{part}")
                sums = mpool.tile([PP, 2], fp32, tag="sums",
                                  name=f"sums_{part}")
                for g in range(2):
                    nc.scalar.activation(probs[:, g, :], sc2[:, g, :], AF.Exp,
                                         accum_out=sums[:, g:g + 1])
                rsum = mpool.tile([PP, 2], fp32, tag="rsum",
                                  name=f"rsum_{part}")
                nc.vector.reciprocal(rsum[:], sums[:])
                for g in range(2):
                    nc.vector.tensor_scalar_mul(probs[:, g, :], probs[:, g, :],
                                                rsum[:, g:g + 1])
                dst = out.rearrange("(p g) c -> p g c", g=2)
                nc.sync.dma_start(dst[part * PP:(part + 1) * PP], probs[:])

            NPARTS = 4
            for mb in range(N_BLKS):
                msl = slice(mb * M_BLK, (mb + 1) * M_BLK)
                if mb == 0:
                    et = et0
                else:
                    et = epool.tile([128, D_TILES, M_BLK], bf16, tag="enc",
                                    name=f"et{mb}")
                    for i in range(8):
                        nc.sync.dma_start(
                            et[:, 2 * i:2 * i + 2, :],
                            encT_t[:, 2 * i:2 * i + 2, msl])
                acc = apool.tile([128, M_BLK], fp32, tag="acc",
                                 name=f"acc{mb}")
                prev_veng = None
                for ht in range(H_TILES):
                    pe = pe_pool.tile([128, M_BLK], fp32, tag="epsum")
                    for dt in range(D_TILES):
                        nc.tensor.matmul(
                            pe[:], wt_sb[ht][:, dt, :], et[:, dt, :],
                            start=(dt == 0), stop=(dt == D_TILES - 1))
                    eng = gpool.tile([128, M_BLK], fp32, tag="eng")
                    nc.scalar.activation(eng[:], pe[:], AF.Tanh,
                                         bias=b_sb[:, ht:ht + 1])
                    veng = vpool.tile([128, M_BLK], fp32, tag="veng",
                                      name=f"veng{mb}_{ht}")
                    nc.scalar.mul(veng[:], eng[:], v_sb[:, ht:ht + 1])
                    # running accumulation: ready ~one ACT after the last MM
                    if ht == 1:
                        nc.vector.tensor_add(acc[:], prev_veng[:], veng[:])
                    elif ht > 1:
                        nc.vector.tensor_add(acc[:], acc[:], veng[:])
                    prev_veng = veng
                # scores[m] = sum over all 1024 h = partition-reduce of acc
                red = apool.tile([128, M_BLK], fp32, tag="red",
                                 name=f"red{mb}")
                nc.gpsimd.partition_all_reduce(red[:], acc[:], 128,
                                               bass_isa.ReduceOp.add)
                nc.sync.dma_start(sc_dram[:, msl], red[0:1, :])
                part = (mb + 1) * NPARTS // N_BLKS - 1
                if part >= 0 and (mb + 1) * NPARTS % N_BLKS == 0 \
                        and part < NPARTS - 1:
                    softmax_part(part, NPARTS)
            softmax_part(NPARTS - 1, NPARTS)

    nc.compile()
    return nc


def kernel(num_features, encoder_outputs, W, b, v):
    global LAST_RESULTS
    from concourse.bass_utils import run_bass_kernel_spmd

    enc = np.asarray(encoder_outputs, dtype=np.float32)
    W_np = np.asarray(W, dtype=np.float32)
    b_np = np.asarray(b, dtype=np.float32)
    v_np = np.asarray(v, dtype=np.float32)
    F = int(np.asarray(num_features))
    assert enc.shape == (L, B, D) and W_np.shape == (H, D)

    # wr[ht, d, j] = W[ht*128 + j, d]
    wr_np = np.ascontiguousarray(
        W_np.reshape(H_TILES, 128, D).transpose(0, 2, 1)).astype(BF16)
    bT_np = np.ascontiguousarray(b_np.reshape(H_TILES, 128).T)     # [128, 8]
    vT_np = np.ascontiguousarray(v_np.ravel().reshape(H_TILES, 128).T)

    in_maps = []
    for c in range(N_CORES):
        shard = enc[c * L_LOC:(c + 1) * L_LOC].reshape(M, D).astype(BF16)
        encT_np = np.ascontiguousarray(shard.T)                    # [D, M]
        in_maps.append({"encT": encT_np, "wr": wr_np, "bT": bT_np,
                        "vT": vT_np})

    if "nc" not in _compiled:
        _compiled["nc"] = _build()
    nc = _compiled["nc"]

    res = run_bass_kernel_spmd(nc, in_maps, core_ids=list(range(N_CORES)))
    LAST_RESULTS = res

    probs = np.concatenate([res.results[c]["out"] for c in range(N_CORES)],
                           axis=0)                                 # [L, B]
    out = np.broadcast_to(probs.T[:, None, :], (B, F, L))
    return np.ascontiguousarray(out)


# revision 6
# speedup vs baseline: 1.3529x; 1.0018x over previous
"""Trainium2 Bass kernel for nn_Attn_69801808495303.

Computes, for encoder_outputs [L, B, 2H], W [H, 2H], b [H], v [H, 1]:
    energy = tanh(enc @ W.T + b)          # [L, B, H]
    scores = energy @ v                   # [L, B]
    attn   = softmax over B (per (L, f))  # broadcast over num_features
    out    = attn as [B, num_features, L]

Strategy: shard over L across 8 NeuronCores (embarrassingly parallel —
the softmax over batch is local to every L row). Host pre-transposes the
encoder shard to [2H, L_loc*B] bf16 so the contraction dim lands on SBUF
partitions; W/b/v are replicated. On device the TensorEngine runs only
the bf16 GEMM (W stationary, energy.T [h, m] tiles in PSUM); ScalarE
applies tanh+bias and the per-partition *v scale; VectorE accumulates the
8 h-tiles; GpSimd reduces over partitions to finish scores = v.tanh(...);
the 64-wide batch softmax runs in quarters so it hides under the GEMM.
Each core returns its [L_loc, B] probability block; the host concatenates
and broadcasts over num_features.
"""

import sys

for _p in ("/opt/trn_rl_repo", "/opt/pypackages"):
    if _p not in sys.path:
        sys.path.append(_p)

import numpy as np
import ml_dtypes

L, B, H, D = 2048, 64, 1024, 2048  # D = 2H
N_CORES = 8
L_LOC = L // N_CORES        # 256 rows of L per core
M = L_LOC * B               # 16384 tokens per core
M_BLK = 512
N_BLKS = M // M_BLK         # 32
D_TILES = D // 128          # 16
H_TILES = H // 128          # 8

BF16 = ml_dtypes.bfloat16

_compiled = {}
LAST_RESULTS = None


def _build():
    import concourse.mybir as mybir
    import concourse.tile as tile
    from concourse import bacc, bass_isa

    fp32, bf16 = mybir.dt.float32, mybir.dt.bfloat16
    AF = mybir.ActivationFunctionType

    nc = bacc.Bacc("TRN2", target_bir_lowering=False, debug=False,
                   num_devices=N_CORES)

    encT = nc.dram_tensor("encT", [D, M], bf16, kind="ExternalInput").ap()
    # weights pre-grouped by h-tile: wr[ht, d, j] = W[ht*128+j, d]
    wr = nc.dram_tensor("wr", [H_TILES, D, 128], bf16,
                        kind="ExternalInput").ap()
    bT = nc.dram_tensor("bT", [128, H_TILES], fp32, kind="ExternalInput").ap()
    vT = nc.dram_tensor("vT", [128, H_TILES], fp32, kind="ExternalInput").ap()
    out = nc.dram_tensor("out", [L_LOC, B], fp32, kind="ExternalOutput").ap()

    encT_t = encT.rearrange("(dt p) m -> p dt m", p=128)  # [128, D_TILES, M]
    wr_t = wr.rearrange("ht (dt p) j -> ht p dt j", p=128)

    with tile.TileContext(nc) as tc:
        with (
            tc.tile_pool(name="const", bufs=1) as cpool,
            tc.tile_pool(name="enc", bufs=32) as epool,
            tc.tile_pool(name="eng", bufs=4) as gpool,
            tc.tile_pool(name="veng", bufs=16) as vpool,
            tc.tile_pool(name="accp", bufs=3) as apool,
            tc.tile_pool(name="misc", bufs=2) as mpool,
            tc.tile_pool(name="psum_e", bufs=6, space="PSUM") as pe_pool,
            tc.tile_pool(name="dram", bufs=1, space="DRAM") as dpool,
        ):
            # Interleave the first et block's chunks with the weight DMAs so
            # the first matmuls start as soon as possible.
            wt_sb = [cpool.tile([128, D_TILES, 128], bf16, name=f"wt{ht}")
                     for ht in range(H_TILES)]

            def load_et(mb):
                msl = slice(mb * M_BLK, (mb + 1) * M_BLK)
                chunks = []
                for i in range(8):
                    ch = epool.tile([128, 2, M_BLK], bf16, tag="enc",
                                    name=f"et{mb}_{i}")
                    nc.sync.dma_start(ch[:], encT_t[:, 2 * i:2 * i + 2, msl])
                    chunks.append(ch)
                return chunks

            nc.sync.dma_start(wt_sb[0][:, 0:8, :], wr_t[0, :, 0:8, :])
            nc.sync.dma_start(wt_sb[0][:, 8:16, :], wr_t[0, :, 8:16, :])
            et0 = load_et(0)
            for ht in range(1, H_TILES):
                nc.sync.dma_start(wt_sb[ht][:, 0:8, :], wr_t[ht, :, 0:8, :])
                nc.sync.dma_start(wt_sb[ht][:, 8:16, :], wr_t[ht, :, 8:16, :])
            b_sb = cpool.tile([128, H_TILES], fp32)
            nc.sync.dma_start(b_sb[:], bT[:])
            v_sb = cpool.tile([128, H_TILES], fp32)
            nc.sync.dma_start(v_sb[:], vT[:])

            sc_dram = dpool.tile([1, M], fp32)

            def softmax_part(part, nparts):
                """Softmax over 64-wide batch groups for one 1/nparts slice
                of the scores (partitions [part*PP, (part+1)*PP) of the
                [128, 2, B] regrouped view)."""
                PP = 128 // nparts
                sc2 = mpool.tile([PP, 2, B], fp32, tag="sc2",
                                 name=f"sc2_{part}")
                src = sc_dram.rearrange("o (p g c) -> (o p) g c", p=128, g=2)
                nc.sync.dma_start(sc2[:], src[part * PP:(part + 1) * PP])
                probs = mpool.tile([PP, 2, B], fp32, tag="probs",
                                   name=f"probs_{part}")
                sums = mpool.tile([PP, 2], fp32, tag="sums",
                                  name=f"sums_{part}")
                for g in range(2):
                    nc.scalar.activation(probs[:, g, :], sc2[:, g, :], AF.Exp,
                                         accum_out=sums[:, g:g + 1])
                rsum = mpool.tile([PP, 2], fp32, tag="rsum",
                                  name=f"rsum_{part}")
                nc.vector.reciprocal(rsum[:], sums[:])
                for g in range(2):
                    nc.vector.tensor_scalar_mul(probs[:, g, :], probs[:, g, :],
                                                rsum[:, g:g + 1])
                dst = out.rearrange("(p g) c -> p g c", g=2)
                nc.sync.dma_start(dst[part * PP:(part + 1) * PP], probs[:])

            def score_block(et, mb, m0, blk, tag):
                """Energy GEMM + tanh + *v + h-sum + partition-reduce for
                tokens [m0, m0+blk); et chunk c holds d-tiles 2c, 2c+1 of
                the block starting at mb*M_BLK (m0 offset within it)."""
                off = m0 - mb * M_BLK
                acc = apool.tile([128, blk], fp32, tag="acc",
                                 name=f"acc{tag}")
                prev_veng = None
                for ht in range(H_TILES):
                    pe = pe_pool.tile([128, blk], fp32, tag="epsum")
                    for dt in range(D_TILES):
                        nc.tensor.matmul(
                            pe[:], wt_sb[ht][:, dt, :],
                            et[dt // 2][:, dt % 2, off:off + blk],
                            start=(dt == 0), stop=(dt == D_TILES - 1))
                    eng = gpool.tile([128, blk], fp32, tag="eng")
                    nc.scalar.activation(eng[:], pe[:], AF.Tanh,
                                         bias=b_sb[:, ht:ht + 1])
                    veng = vpool.tile([128, blk], fp32, tag="veng",
                                      name=f"veng{tag}_{ht}")
                    nc.scalar.mul(veng[:], eng[:], v_sb[:, ht:ht + 1])
                    # running accumulation: ready ~one ACT after the last MM
                    if ht == 1:
                        nc.vector.tensor_add(acc[:], prev_veng[:], veng[:])
                    elif ht > 1:
                        nc.vector.tensor_add(acc[:], acc[:], veng[:])
                    prev_veng = veng
                # scores[m] = sum over all 1024 h = partition-reduce of acc
                red = apool.tile([128, blk], fp32, tag="red",
                                 name=f"red{tag}")
                nc.gpsimd.partition_all_reduce(red[:], acc[:], 128,
                                               bass_isa.ReduceOp.add)
                nc.sync.dma_start(sc_dram[:, m0:m0 + blk], red[0:1, :])

            NPARTS = 4
            for mb in range(N_BLKS):
                et = et0 if mb == 0 else load_et(mb)
                if mb == N_BLKS - 1:
                    # split the last block so the final score chain (which
                    # nothing hides) is short
                    for sb in range(2):
                        score_block(et, mb, mb * M_BLK + sb * (M_BLK // 2),
                                    M_BLK // 2, f"{mb}_{sb}")
                else:
                    score_block(et, mb, mb * M_BLK, M_BLK, str(mb))
                part = (mb + 1) * NPARTS // N_BLKS - 1
                if part >= 0 and (mb + 1) * NPARTS % N_BLKS == 0 \
                        and part < NPARTS - 1:
                    softmax_part(part, NPARTS)
            softmax_part(NPARTS - 1, NPARTS)

    nc.compile()
    return nc


def kernel(num_features, encoder_outputs, W, b, v):
    global LAST_RESULTS
    from concourse.bass_utils import run_bass_kernel_spmd

    enc = np.asarray(encoder_outputs, dtype=np.float32)
    W_np = np.asarray(W, dtype=np.float32)
    b_np = np.asarray(b, dtype=np.float32)
    v_np = np.asarray(v, dtype=np.float32)
    F = int(np.asarray(num_features))
    assert enc.shape == (L, B, D) and W_np.shape == (H, D)

    # wr[ht, d, j] = W[ht*128 + j, d]
    wr_np = np.ascontiguousarray(
        W_np.reshape(H_TILES, 128, D).transpose(0, 2, 1)).astype(BF16)
    bT_np = np.ascontiguousarray(b_np.reshape(H_TILES, 128).T)     # [128, 8]
    vT_np = np.ascontiguousarray(v_np.ravel().reshape(H_TILES, 128).T)

    in_maps = []
    for c in range(N_CORES):
        shard = enc[c * L_LOC:(c + 1) * L_LOC].reshape(M, D).astype(BF16)
        encT_np = np.ascontiguousarray(shard.T)                    # [D, M]
        in_maps.append({"encT": encT_np, "wr": wr_np, "bT": bT_np,
                        "vT": vT_np})

    if "nc" not in _compiled:
        _compiled["nc"] = _build()
    nc = _compiled["nc"]

    res = run_bass_kernel_spmd(nc, in_maps, core_ids=list(range(N_CORES)))
    LAST_RESULTS = res

    probs = np.concatenate([res.results[c]["out"] for c in range(N_CORES)],
                           axis=0)                                 # [L, B]
    out = np.broadcast_to(probs.T[:, None, :], (B, F, L))
    return np.ascontiguousarray(out)
